# revision 1
# baseline (speedup 1.0000x reference)
"""Causal multi-head attention block (qkv proj + partial RoPE + causal attn +
out proj) for Trainium2, distributed over 8 NeuronCores.

Sharding: core i handles batch b = i//2 and head-group g = i%2 (6 of 12 heads).
Each core computes a partial output projection (contraction over its 6 heads'
384 channels); the host sums the two head-group partials per batch.

v2 design notes:
  - Projections (qkv, out) in float32r (full-rate fp32): ~1.5e-4 relative.
  - Attention core (scores, attn@v) in bf16: fp32r's fused 4-byte weight load
    serializes LDW+MM and defeats tile_position row-pair overlap; bf16 gets
    pipelined LDW + true packing. Scores feed exp whose output is softmax
    weights -- bf16 there costs ~1e-3 relative worst case.
  - Phases interleaved per 512-token tile jt: project(jt) -> attention(jq=jt)
    (causal: q-tile jt only needs k-tiles <= jt) -> out-projection(jt).
  - PSUM: scores pool [128,2,512]x2 (4 banks) + o accumulators [128,2,512]x1
    (2 banks) + shared flex pool (2 banks) for proj/outproj/broadcast.
  - Softmax: no max-subtraction needed (|scores/8| < ~3), denominator via a
    ones-column in v (o_ext row 64), normalization decoupled from the PSUM o
    slot: rowsums copied to SBUF by ACT, DVE reciprocal, K=33 f32r broadcast
    matmul, DVE multiply-evict.
"""

import numpy as np

B, T, C = 4, 2048, 768
NH, HD, RD = 12, 64, 16
NHL = NH // 2          # heads per core (local)
NPAIR = NHL // 2       # head pairs per core
CL = NHL * HD          # local channels (384)
TQ = 512               # q tile
NTQ = T // TQ
NKT = T // 128         # k tiles of 128

_cache = {}


def _build(debug=False):
    import concourse.bacc as bacc
    import concourse.mybir as mybir
    import concourse.tile as tile

    F32R = mybir.dt.float32r
    F32 = mybir.dt.float32
    BF16 = mybir.dt.bfloat16
    AF = mybir.ActivationFunctionType
    MUL = mybir.AluOpType.mult
    SUB = mybir.AluOpType.subtract
    ADD = mybir.AluOpType.add

    nc = bacc.Bacc(trn_type="TRN2", name="attn8")

    xt = nc.dram_tensor("xt", [C, T], F32R, kind="ExternalInput")
    wqkt = nc.dram_tensor("wqkt", [C, 2 * CL], F32R, kind="ExternalInput")
    wvt = nc.dram_tensor("wvt", [C, CL], F32R, kind="ExternalInput")
    wot = nc.dram_tensor("wot", [CL, C], F32R, kind="ExternalInput")
    cosb = nc.dram_tensor("cosb", [96, T], F32, kind="ExternalInput")
    sinb = nc.dram_tensor("sinb", [96, T], F32, kind="ExternalInput")
    tri = nc.dram_tensor("tri", [128, 128], BF16, kind="ExternalInput")
    e6 = nc.dram_tensor("e6", [6, NPAIR * 128], F32R, kind="ExternalInput")
    out = nc.dram_tensor("out", [C, T], F32, kind="ExternalOutput")
    if debug:
        dbg_qk = nc.dram_tensor("dbg_qk", [128, 2 * NPAIR, T], F32,
                                kind="ExternalOutput")
        dbg_v = nc.dram_tensor("dbg_v", [128, NKT, NHL, HD + 1], F32,
                               kind="ExternalOutput")
        dbg_o = nc.dram_tensor("dbg_o", [128, NPAIR, T], F32,
                               kind="ExternalOutput")

    # qk-projection M-tiles (wqkt column order, host-built):
    #   tile 0: r1 rows [96] = (q h0..h5 | k h0..h5) x dims 0:8
    #   tile 1: r2 rows [96] = same x dims 8:16
    #   tiles 2..6: pass [128,128,128,128,64] = (q h0..h5 | k h0..h5) x dims 16:64
    MT_SIZES = [96, 96, 128, 128, 128, 128, 64]
    MT_OFF = np.cumsum([0] + MT_SIZES).tolist()

    def pass_dest(row):
        a, r = divmod(row, 48)        # a: tensor-head 0..11 (q first), r: dim-16
        tn, hl = divmod(a, NHL)
        blk = (0 if tn == 0 else NPAIR) + hl // 2
        part = 64 * (hl % 2) + 16 + r
        return blk, part

    with tile.TileContext(nc) as tc:
        with (
            tc.tile_pool(name="persist", bufs=1) as pp,
            tc.tile_pool(name="weights", bufs=1) as wp,
            tc.tile_pool(name="xload", bufs=2) as xlp,
            tc.tile_pool(name="pstage", bufs=2) as psg,
            tc.tile_pool(name="ropet", bufs=1) as rtp,
            tc.tile_pool(name="expp", bufs=3) as xpp,
            tc.tile_pool(name="misc", bufs=2) as msc,
            tc.tile_pool(name="onorm", bufs=3) as onp,
            tc.tile_pool(name="flex", bufs=2, space="PSUM") as flx,
            tc.tile_pool(name="sps", bufs=2, space="PSUM") as sps,
            tc.tile_pool(name="ops", bufs=1, space="PSUM") as ops,
        ):
            qk_sb = pp.tile([128, 2 * NPAIR, T], BF16, tag="qk")
            v_sb = pp.tile([128, NKT, NHL, HD + 1], BF16, tag="v")
            o_sb = pp.tile([128, NPAIR, T], F32R, tag="o")
            cos_t = pp.tile([96, T], F32, tag="cos")
            sin_t = pp.tile([96, T], F32, tag="sin")
            tri_t = pp.tile([128, 128], BF16, tag="tri")
            e6_t = pp.tile([6, NPAIR * 128], F32R, tag="e6")
            rs6_t = pp.tile([6, TQ], F32, tag="rs6")
            rinv6_t = pp.tile([6, TQ], F32R, tag="rinv6")
            rot1 = pp.tile([96, T], BF16, tag="rot1")
            rot2 = pp.tile([96, T], BF16, tag="rot2")

            wqk_t = wp.tile([128, C // 128, 2 * CL], F32R, tag="wqk")
            wv_t = wp.tile([128, C // 128, CL], F32R, tag="wv")
            wo_t = wp.tile([128, NPAIR, C], F32R, tag="wo")

            nc.sync.dma_start(cos_t, cosb[:, :])
            nc.sync.dma_start(sin_t, sinb[:, :])
            nc.sync.dma_start(tri_t, tri[:, :])
            nc.sync.dma_start(e6_t, e6[:, :])
            nc.gpsimd.memset(
                v_sb.bitcast(mybir.dt.uint16).rearrange("p a b c -> p (a b c)"),
                0x3F80)  # bf16 1.0 bit pattern
            wqk_r = wqkt.rearrange("(co p) m -> co p m", p=128)
            for c in range(C // 128):
                # rope columns first so the first M-tiles' matmuls start early
                nc.sync.dma_start(wqk_t[:, c, 0:192], wqk_r[c, :, 0:192])
                nc.sync.dma_start(wqk_t[:, c, 192:2 * CL], wqk_r[c, :, 192:2 * CL])
                nc.sync.dma_start(
                    wv_t[:, c], wvt.rearrange("(co p) m -> co p m", p=128)[c])
            for p in range(NPAIR):
                nc.sync.dma_start(
                    wo_t[:, p], wot.rearrange("(po p) m -> po p m", p=128)[p])

            for jt in range(NTQ):
                ts = slice(jt * TQ, (jt + 1) * TQ)
                # ---------------- projections for this t-tile ----------------
                x_jt = xlp.tile([128, C // 128, TQ], F32R, tag="x")
                for c in range(C // 128):
                    nc.sync.dma_start(
                        x_jt[:, c],
                        xt.rearrange("(co p) t -> co p t", p=128)[c, :, ts])

                ps_r = sps.tile([128, 2, TQ], F32, tag="s")
                for mt in range(2):
                    R = MT_SIZES[mt]
                    for c in range(C // 128):
                        nc.tensor.matmul(
                            ps_r[0:R, mt, :], wqk_t[:, c, MT_OFF[mt]:MT_OFF[mt + 1]],
                            x_jt[:, c], start=(c == 0), stop=(c == C // 128 - 1))
                # rope: rot1 = r1*cos - r2*sin ; rot2 = r2*cos + r1*sin
                t1 = rtp.tile([96, TQ], F32, tag="t1")
                t2 = rtp.tile([96, TQ], F32, tag="t2")
                nc.vector.tensor_tensor(t1, ps_r[0:96, 0, :], cos_t[:, ts], MUL)
                nc.vector.tensor_tensor(t2, ps_r[0:96, 1, :], sin_t[:, ts], MUL)
                nc.vector.tensor_tensor(rot1[:, ts], t1, t2, SUB)
                t3 = rtp.tile([96, TQ], F32, tag="t1")
                t4 = rtp.tile([96, TQ], F32, tag="t2")
                nc.vector.tensor_tensor(t3, ps_r[0:96, 1, :], cos_t[:, ts], MUL)
                nc.vector.tensor_tensor(t4, ps_r[0:96, 0, :], sin_t[:, ts], MUL)
                nc.vector.tensor_tensor(rot2[:, ts], t3, t4, ADD)
                for a in range(12):
                    tn, hl = divmod(a, NHL)
                    blk = (0 if tn == 0 else NPAIR) + hl // 2
                    base = 64 * (hl % 2)
                    nc.sync.dma_start(qk_sb[base:base + 8, blk, ts],
                                      rot1[8 * a:8 * a + 8, ts])
                    nc.sync.dma_start(qk_sb[base + 8:base + 16, blk, ts],
                                      rot2[8 * a:8 * a + 8, ts])

                for mt in range(2, 7):
                    R = MT_SIZES[mt]
                    ps = flx.tile([128, TQ], F32, tag="flex")
                    for c in range(C // 128):
                        nc.tensor.matmul(
                            ps[0:R], wqk_t[:, c, MT_OFF[mt]:MT_OFF[mt + 1]],
                            x_jt[:, c], start=(c == 0), stop=(c == C // 128 - 1))
                    stg = psg.tile([128, TQ], BF16, tag="pstg")
                    nc.vector.tensor_copy(stg[0:R], ps[0:R])
                    row0 = MT_OFF[mt] - MT_OFF[2]
                    row = row0
                    while row < row0 + R:
                        blk, part = pass_dest(row)
                        run = min(row0 + R - row, 48 - row % 48)
                        nc.sync.dma_start(
                            qk_sb[part:part + run, blk, ts],
                            stg[row - row0:row - row0 + run])
                        row += run

                for vt in range(TQ // 128):
                    pvf = flx.tile([128, TQ], F32, tag="flex")
                    pv = pvf[:, 0:CL]
                    kt0 = jt * (TQ // 128) + vt
                    for c in range(C // 128):
                        nc.tensor.matmul(
                            pv, x_jt[:, c, vt * 128:(vt + 1) * 128],
                            wv_t[:, c], start=(c == 0), stop=(c == C // 128 - 1))
                    nc.vector.tensor_copy(
                        v_sb[:, kt0, :, 0:HD],
                        pv.rearrange("p (h d) -> p h d", d=HD))

            # ---------------- attention (de-interleaved test) ---------------
            for jq in range(NTQ):
                qs = slice(jq * TQ, (jq + 1) * TQ)
                ouns = []
                for p in range(NPAIR):
                    qb = qk_sb[:, p, qs]
                    kb = qk_sb[:, NPAIR + p, :]
                    o_ps = ops.tile([128, 2, TQ], F32, tag="o")
                    nkt = 4 * (jq + 1)
                    for kt in range(nkt):
                        m = kt - 4 * jq
                        a = 0 if m < 0 else 128 * m
                        ks = slice(kt * 128, (kt + 1) * 128)
                        sg = sps.tile([128, 2, TQ], F32, tag="s")
                        nc.tensor.matmul(
                            sg[:, 0, a:TQ], kb[0:64, ks], qb[0:64, a:TQ],
                            start=True, stop=True, tile_position=(0, 0))
                        nc.tensor.matmul(
                            sg[:, 1, a:TQ], kb[64:128, ks], qb[64:128, a:TQ],
                            start=True, stop=True, tile_position=(64, 0))
                        ep = xpp.tile([128, 2, TQ], BF16, tag="e")
                        nc.scalar.activation(ep[:, :, a:TQ], sg[:, :, a:TQ],
                                             AF.Exp, scale=0.125)
                        if m >= 0:
                            for h in range(2):
                                nc.gpsimd.tensor_tensor(
                                    ep[:, h, a:a + 128],
                                    ep[:, h, a:a + 128], tri_t, MUL)
                        for h in range(2):
                            nc.tensor.matmul(
                                o_ps[0:65, h, a:TQ],
                                v_sb[:, kt, 2 * p + h, :], ep[:, h, a:TQ],
                                start=(kt == 0), stop=(kt == nkt - 1))
                    # decoupled softmax normalization:
                    # rowsums -> SBUF (ACT), o -> SBUF unnormalized (ACT),
                    # then DVE reciprocal + K=33 broadcast + DVE mul-evict.
                    oun = onp.tile([128, 2, TQ], F32, tag="oun")
                    nc.scalar.copy(oun[0:65, :, :], o_ps[0:65, :, :])
                    for h in range(2):
                        nc.sync.dma_start(rs6_t[2 * p + h:2 * p + h + 1, :],
                                          oun[64:65, h, :])
                    ouns.append(oun)
                # batched softmax denominators for all 3 pairs of this q-tile
                with nc.allow_low_precision(reason="f32r storage is 32-bit"):
                    nc.vector.reciprocal(rinv6_t, rs6_t)
                for p in range(NPAIR):
                    bc = flx.tile([128, TQ], F32, tag="flex")
                    nc.tensor.matmul(bc, e6_t[:, p * 128:(p + 1) * 128], rinv6_t,
                                     start=True, stop=True)
                    oun = ouns[p]
                    nc.vector.tensor_tensor(
                        o_sb[0:64, p, qs], oun[0:64, 0, :], bc[0:64], MUL)
                    nc.vector.tensor_tensor(
                        o_sb[64:128, p, qs], oun[0:64, 1, :], bc[64:128], MUL)

                # ---------------- output projection for this t-tile ----------
                for dt in range(C // 128):
                    po = flx.tile([128, TQ], F32, tag="flex")
                    for p in range(NPAIR):
                        nc.tensor.matmul(
                            po, wo_t[:, p, dt * 128:(dt + 1) * 128],
                            o_sb[:, p, qs], start=(p == 0), stop=(p == NPAIR - 1))
                    ost = msc.tile([128, TQ], F32, tag="ost")
                    nc.vector.tensor_copy(ost, po)
                    nc.sync.dma_start(
                        out.rearrange("(do p) t -> do p t", p=128)[dt, :, qs], ost)

            if debug:
                with tc.tile_pool(name="dbgp", bufs=2) as dbp:
                    for blk in range(2 * NPAIR):
                        dcp = dbp.tile([128, T], F32, tag="dbgc")
                        nc.scalar.copy(dcp, qk_sb[:, blk, :])
                        nc.sync.dma_start(dbg_qk[:, blk, :], dcp)
                    for p in range(NPAIR):
                        dcp = dbp.tile([128, T], F32, tag="dbgc")
                        nc.scalar.copy(dcp, o_sb[:, p, :])
                        nc.sync.dma_start(dbg_o[:, p, :], dcp)
                    for kt in range(NKT):
                        dcv = dbp.tile([128, NHL * (HD + 1)], F32, tag="dbgc")
                        nc.scalar.copy(
                            dcv[:, 0:NHL * (HD + 1)],
                            v_sb[:, kt].rearrange("p b c -> p (b c)"))
                        nc.sync.dma_start(
                            dbg_v[:, kt].rearrange("p b c -> p (b c)"),
                            dcv[:, 0:NHL * (HD + 1)])

    nc.compile()
    return nc


def _host_inputs(x, w_qkv, w_out):
    """Build per-core input dicts. Core i: batch i//2, head-group i%2."""
    import ml_dtypes

    xf = np.ascontiguousarray(x, dtype=np.float32)
    w3 = np.asarray(w_qkv, dtype=np.float32).reshape(3, NH, HD, C)
    wo = np.asarray(w_out, dtype=np.float32)

    per_group = []
    for g in range(2):
        hs = range(g * NHL, (g + 1) * NHL)
        rows = []
        for dd0, dd1 in ((0, 8), (8, 16)):
            for tn in range(2):
                for h in hs:
                    rows.append(w3[tn, h, dd0:dd1])         # [8, C]
        for tn in range(2):
            for h in hs:
                rows.append(w3[tn, h, 16:64])               # [48, C]
        wqk = np.concatenate(rows, axis=0)                  # [768, C]
        wqkt = np.ascontiguousarray(wqk.T)                  # [C, 768]
        wv = w3[2, list(hs)].reshape(CL, C)                 # [384, C]
        wvt = np.ascontiguousarray(wv.T)
        wotr = np.ascontiguousarray(wo[:, g * CL:(g + 1) * CL].T)  # [384, 768]
        per_group.append((wqkt, wvt, wotr))

    j = np.arange(RD // 2, dtype=np.float64)
    freqs = 1.0 / (10000.0 ** (2 * j / RD))
    t = np.arange(T, dtype=np.float64)
    ang = t[None, :] * freqs[:, None]                        # [8, T]
    cosb = np.ascontiguousarray(np.tile(np.cos(ang), (12, 1)), dtype=np.float32)
    sinb = np.ascontiguousarray(np.tile(np.sin(ang), (12, 1)), dtype=np.float32)

    kk = np.arange(128)[:, None]
    qq = np.arange(128)[None, :]
    tri = (kk <= qq).astype(ml_dtypes.bfloat16)
    e6 = np.zeros((6, NPAIR * 128), dtype=np.float32)
    for p in range(NPAIR):
        e6[2 * p, p * 128:p * 128 + 64] = 1.0
        e6[2 * p + 1, p * 128 + 64:(p + 1) * 128] = 1.0

    in_maps = []
    for i in range(8):
        b, g = divmod(i, 2)
        wqkt, wvt, wotr = per_group[g]
        in_maps.append({
            "xt": np.ascontiguousarray(xf[b].T),
            "wqkt": wqkt, "wvt": wvt, "wot": wotr,
            "cosb": cosb, "sinb": sinb, "tri": tri, "e6": e6,
        })
    return in_maps


def kernel(x, w_qkv, w_out, _trace=False):
    from concourse.bass_utils import run_bass_kernel_spmd

    if "nc" not in _cache:
        _cache["nc"] = _build()
    nc = _cache["nc"]
    in_maps = _host_inputs(x, w_qkv, w_out)
    res = run_bass_kernel_spmd(nc, in_maps, core_ids=list(range(8)),
                               trace=_trace)
    _cache["last_result"] = res
    out = np.empty((B, T, C), dtype=np.float32)
    for b in range(B):
        acc = res.results[2 * b]["out"].astype(np.float32) + \
            res.results[2 * b + 1]["out"].astype(np.float32)
        out[b] = acc.T
    return out

